# revision 8
# baseline (speedup 1.0000x reference)
"""AttentionFreeTransformer kernel for 8 TRN2 NeuronCores.

Reference computation (B=4, T=4096, D=2048):
    qkv = rmsnorm(x) @ w_qkv.T            # [B, T, 3D]
    q, k, v = split(qkv)
    q = rmsnorm(q); k = rmsnorm(k)
    w = exp(k); kv = w * v
    y = cumsum(kv, T) / (cumsum(w, T) + 1e-6)
    out = (x, sigmoid(q) * y)

Sharding: core = 2*b + h owns batch b, sequence half h (TL=2048 tokens).
All device tensors live in transposed layout [channel partitions, token free]
so the T-cumsum is a DVE tensor_tensor_scan along the free axis and the
cross-core carry (first-half column sums -> second-half core) is the scan's
per-partition `initial`, exchanged with one 16KB pairwise AllReduce.

Algebraic notes:
  - rmsnorm(x)'s per-token scale inv_x factors out of the projection:
    qkv_ref = inv_x[t] * (x @ w.T).  q and k are re-rmsnormed, which cancels
    the inv_x factor (up to eps), so only v needs inv_x applied.
  - rsqrt and reciprocal are computed as exp(-0.5*ln(.)) / exp(-ln(.)) on the
    scalar engine (natural_log_exp table set; Rsqrt/Reciprocal ACT funcs are
    banned for accuracy, DVE reciprocal is 8 cyc/elem).
"""

import sys

sys.path.insert(0, "/opt/trn_rl_repo")

import numpy as np
import ml_dtypes

import concourse.bass as bass
import concourse.bacc as bacc_mod
import concourse.mybir as mybir
from concourse.bass import ds, ts
from concourse.tile import TileContext

BF16 = ml_dtypes.bfloat16
F32EPS = float(np.finfo(np.float32).eps)

B, T, D = 4, 4096, 2048
NCORES = 8
TL = T // 2  # tokens per core

AF = mybir.ActivationFunctionType
ALU = mybir.AluOpType


class _Bacc(bacc_mod.Bacc):
    """Bacc whose act-table chooser maps all our funcs to one set.

    The default chooser assigns each activation to the first table set
    containing its func, which alternates exp_and_others / natural_log for
    interleaved Exp/Ln and costs ~2.7us per ACT_TABLE_LOAD. Filtering the
    candidate funcs (set indices preserved -- walrus maps id -> its own
    act_info.json) forces Exp/Ln/Square/Copy -> natural_log_exp_and_others
    and Sigmoid -> sigmoid_and_others: 2 loads total."""

    def insert_act_table_loads(self):
        from concourse.hw_specs import get_activation_tables
        from concourse.bacc import _bass_rust

        has_activation = any(
            isinstance(i, mybir.InstActivation)
            for b in self.main_func.blocks
            for i in b.instructions
        )
        if not has_activation:
            return
        ours = {AF.Exp, AF.Ln, AF.Square, AF.Copy, AF.Identity, AF.Sigmoid}
        tables = []
        for name, funcs in get_activation_tables(self.m.arch).items():
            if name == "natural_log_exp_and_others":
                tables.append((name, funcs))
            elif name == "sigmoid_and_others":
                tables.append((name, (funcs - ours) | {AF.Sigmoid}))
            else:
                tables.append((name, funcs - ours))
        _bass_rust.insert_act_table_loads(self, tables)


def build_kernel(D_=D, TL_=TL, n_cores=NCORES):
    P = 128
    ND = D_ // P          # channel subtiles (16)
    NT = TL_ // 512       # 512-token psum chunks (4)
    E_BLKS = 3 * ND       # e tiles across q|k|v
    inv_scale = 1.0 / D_

    nc = _Bacc(target_bir_lowering=False, num_devices=n_cores)

    f32 = mybir.dt.float32
    bf16 = mybir.dt.bfloat16

    xT_h = nc.declare_dram_parameter("xT", [P, ND, TL_], bf16, isOutput=False)
    wT_h = nc.declare_dram_parameter("wT", [E_BLKS, P, ND, P], bf16, isOutput=False)
    cmask_h = nc.declare_dram_parameter("cmask", [P, 1], f32, isOutput=False)
    smask_h = nc.declare_dram_parameter("smask", [P, 1], f32, isOutput=False)
    out_h = nc.declare_dram_parameter("outT", [ND, P, TL_], bf16, isOutput=True)

    ones_col_h = nc.inline_tensor(np.ones((P, 1), dtype=BF16), name="ones_col")
    ones_row_h = nc.inline_tensor(np.ones((1, P), dtype=BF16), name="ones_row")

    groups = [[i, i + 1] for i in range(0, n_cores, 2)]

    with (
        TileContext(nc) as tc,
        tc.tile_pool(name="const", bufs=1) as const,
        tc.tile_pool(name="wstream", bufs=3) as wstream,
        tc.tile_pool(name="scr32", bufs=3) as scr32,
        tc.tile_pool(name="scr16", bufs=5) as scr16,
        tc.tile_pool(name="scrB16", bufs=10) as scrB16,
        tc.tile_pool(name="scrB32", bufs=3) as scrB32,
        tc.tile_pool(name="rows", bufs=1) as rows,
        tc.tile_pool(name="mmps", bufs=6, space="PSUM") as mmps,
        tc.tile_pool(name="ssqps", bufs=2, space="PSUM") as ssqps,
        tc.tile_pool(name="spill", bufs=1, space="DRAM") as spill,
    ):
        # ---- resident tiles ----
        xT_sb = const.tile([P, ND, TL_], bf16, tag="xT_sb")
        for do in range(ND):
            nc.sync.dma_start(out=xT_sb[:, do, :], in_=xT_h[:, do, :])

        ones_col = const.tile([P, 1], bf16, tag="ones_col")
        nc.sync.dma_start(out=ones_col[:], in_=ones_col_h[:])
        ones_row = const.tile([1, P], bf16, tag="ones_row")
        nc.sync.dma_start(out=ones_row[:], in_=ones_row_h[:])
        cmask = const.tile([P, 1], f32, tag="cmask")
        nc.sync.dma_start(out=cmask[:], in_=cmask_h[:])
        smask = const.tile([P, 1], f32, tag="smask")
        nc.sync.dma_start(out=smask[:], in_=smask_h[:])

        eps_b = const.tile([P, 1], f32, tag="eps_b")
        nc.vector.memset(eps_b[:], F32EPS)
        eps6_b = const.tile([P, 1], f32, tag="eps6_b")
        nc.vector.memset(eps6_b[:], 1e-6)

        inv_x = const.tile([P, TL_], bf16, tag="inv_x")
        inv_k = const.tile([P, TL_], bf16, tag="inv_k")
        inv_q = const.tile([P, TL_], bf16, tag="inv_q")
        carry_both = const.tile([P, 2 * ND], f32, tag="carry_both")
        carry_use = const.tile([P, 2 * ND], f32, tag="carry_use")

        # ---- DRAM spill arrays ----
        q_sp = spill.tile([ND, P, TL_], bf16, tag="q_sp")
        k_sp = spill.tile([ND, P, TL_], bf16, tag="k_sp")
        w_sp = spill.tile([ND, P, TL_], bf16, tag="w_sp")
        kv_sp = spill.tile([ND, P, TL_], bf16, tag="kv_sp")
        y_sp = spill.tile([ND, P, TL_], bf16, tag="y_sp")
        cc_in = spill.tile([P, 2 * ND], f32, tag="cc_in")
        cc_out = spill.tile([P, 2 * ND], f32, tag="cc_out")

        def inv_chain(ssq_tiles, dest, extra_scale):
            """dest[p, t] = (ssq[t]/D + eps) ** -0.5 for all p (replicated).

            ssq_tiles: NT psum tiles [1, 512] holding per-token sums of squares.
            """
            row = rows.tile([1, TL_], bf16, tag="row")
            for tci in range(NT):
                bank, rp = divmod(tci, 2)
                nc.scalar.copy(
                    out=row[:, ts(tci, 512)],
                    in_=ssq_tiles[bank][32 * rp : 32 * rp + 1, :],
                )
            lnv = scr32.tile([P, TL_], f32, tag="s32")
            for tci in range(NT):
                rep = ssqps.tile([P, 512], f32, tag="ssq", name="rep")
                nc.tensor.matmul(
                    out=rep[:],
                    lhsT=ones_row[:],
                    rhs=row[:, ts(tci, 512)],
                    start=True,
                    stop=True,
                )
                nc.scalar.activation(
                    lnv[:, ts(tci, 512)], rep[:], AF.Ln,
                    bias=eps_b[:], scale=inv_scale,
                )
            nc.scalar.activation(dest[:], lnv[:], AF.Exp, scale=extra_scale)

        def projection(e_blk_base, j, psum_tiles):
            """One 128-wide output-channel tile of qkv = wT.T @ xT."""
            wsb = wstream.tile([P, ND, P], bf16, tag="wsb")
            nc.sync.dma_start(out=wsb[:], in_=wT_h[e_blk_base + j])
            for tci in range(NT):
                for do in range(ND):
                    nc.tensor.matmul(
                        out=psum_tiles[tci][:],
                        lhsT=wsb[:, do, :],
                        rhs=xT_sb[:, do, ts(tci, 512)],
                        start=(do == 0),
                        stop=(do == ND - 1),
                    )

        def ssq_accumulate(src_sb, ssq_tiles, j):
            """Accumulate per-token sum of squares of src_sb into ssq psum."""
            sq = scr16.tile([P, TL_], bf16, tag="s16")
            nc.scalar.activation(sq[:], src_sb[:], AF.Square)
            for tci in range(NT):
                bank, rp = divmod(tci, 2)
                nc.tensor.matmul(
                    out=ssq_tiles[bank][32 * rp : 32 * rp + 1, :],
                    lhsT=ones_col[:],
                    rhs=sq[:, ts(tci, 512)],
                    start=(j == 0),
                    stop=(j == ND - 1),
                )

        # ---- phase K: k projection, spill, ssq (runs first so PE starts hot) ----
        kssq = [ssqps.tile([64, 512], f32, tag="ssq", name=f"kssq{i}") for i in range((NT + 1) // 2)]
        for j in range(ND):
            pk = [mmps.tile([P, 512], f32, tag="mm", name=f"pk{j}_{i}") for i in range(NT)]
            projection(ND, j, pk)
            ksb = scr16.tile([P, TL_], bf16, tag="s16")
            for tci in range(NT):
                nc.scalar.copy(out=ksb[:, ts(tci, 512)], in_=pk[tci][:])
            nc.gpsimd.dma_start(out=k_sp[j], in_=ksb[:])
            ssq_accumulate(ksb, kssq, j)
        inv_chain(kssq, inv_k, -0.5)

        # ---- phase X: sum of squares of x, inv_x (hidden under phase K) ----
        xssq = [ssqps.tile([64, 512], f32, tag="ssq", name=f"xssq{i}") for i in range((NT + 1) // 2)]
        for do in range(ND):
            sq = scr16.tile([P, TL_], bf16, tag="s16")
            nc.scalar.activation(sq[:], xT_sb[:, do, :], AF.Square)
            for tci in range(NT):
                bank, rp = divmod(tci, 2)
                nc.tensor.matmul(
                    out=xssq[bank][32 * rp : 32 * rp + 1, :],
                    lhsT=ones_col[:],
                    rhs=sq[:, ts(tci, 512)],
                    start=(do == 0),
                    stop=(do == ND - 1),
                )
        inv_chain(xssq, inv_x, -0.5)

        # ---- phase V + 4a: v projection, w=exp(k*invk), kv, totals ----
        for c in range(ND):
            pv = [mmps.tile([P, 512], f32, tag="mm", name=f"pv{c}_{i}") for i in range(NT)]
            projection(2 * ND, c, pv)
            vsb = scr32.tile([P, TL_], f32, tag="s32")
            for tci in range(NT):
                nc.vector.tensor_mul(
                    out=vsb[:, ts(tci, 512)], in0=pv[tci][:],
                    in1=inv_x[:, ts(tci, 512)],
                )
            kc = scr16.tile([P, TL_], bf16, tag="s16")
            nc.sync.dma_start(out=kc[:], in_=k_sp[c])
            kn = scr32.tile([P, TL_], f32, tag="s32")
            nc.vector.tensor_mul(out=kn[:], in0=kc[:], in1=inv_k[:])
            wc = scr16.tile([P, TL_], bf16, tag="s16")
            nc.scalar.activation(
                wc[:], kn[:], AF.Exp, accum_out=carry_both[:, c : c + 1]
            )
            kvc = scr16.tile([P, TL_], bf16, tag="s16")
            nc.vector.scalar_tensor_tensor(
                out=kvc[:], in0=wc[:], scalar=1.0, in1=vsb[:],
                op0=ALU.mult, op1=ALU.mult,
                accum_out=carry_both[:, ND + c : ND + c + 1],
            )
            nc.gpsimd.dma_start(out=w_sp[c], in_=wc[:])
            nc.gpsimd.dma_start(out=kv_sp[c], in_=kvc[:])

        # ---- carry exchange send: even core's totals -> pairwise AllReduce ----
        snd = scr32.tile([P, 2 * ND], f32, tag="snd")
        nc.vector.tensor_scalar_mul(snd[:], carry_both[:], smask[:])
        nc.gpsimd.dma_start(out=cc_in[:], in_=snd[:])
        nc.gpsimd.collective_compute(
            "AllReduce",
            ALU.add,
            replica_groups=groups,
            ins=[cc_in[:]],
            outs=[cc_out[:]],
        )

        # ---- phase Q: q projection, spill, ssq ----
        # The collective-result receive is issued AFTER j==2's weight loads so
        # it cannot head-of-line-block the Sync DMA queue that feeds the
        # projection's wT stream (the baseline stalled the PE ~23us here).
        qssq = [ssqps.tile([64, 512], f32, tag="ssq", name=f"qssq{i}") for i in range((NT + 1) // 2)]
        for j in range(ND):
            pq = [mmps.tile([P, 512], f32, tag="mm", name=f"pq{j}_{i}") for i in range(NT)]
            projection(0, j, pq)
            qsb = scr16.tile([P, TL_], bf16, tag="s16")
            for tci in range(NT):
                nc.scalar.copy(out=qsb[:, ts(tci, 512)], in_=pq[tci][:])
            nc.gpsimd.dma_start(out=q_sp[j], in_=qsb[:])
            ssq_accumulate(qsb, qssq, j)
            if j == min(2, ND - 1):
                rcv = scr32.tile([P, 2 * ND], f32, tag="snd")
                nc.sync.dma_start(out=rcv[:], in_=cc_out[:])
                nc.vector.tensor_scalar_mul(carry_use[:], rcv[:], cmask[:])
        inv_chain(qssq, inv_q, -0.5)

        # ---- phase 4bA: scans, y = cumsum(kv)/(cumsum(w)+1e-6) ----
        for c in range(ND):
            wcl = scrB16.tile([P, TL_], bf16, tag="b16")
            nc.sync.dma_start(out=wcl[:], in_=w_sp[c])
            kvl = scrB16.tile([P, TL_], bf16, tag="b16")
            nc.sync.dma_start(out=kvl[:], in_=kv_sp[c])
            wcum = scrB16.tile([P, TL_], bf16, tag="b16")
            nc.vector.tensor_tensor_scan(
                out=wcum[:], data0=wcl[:], data1=wcl[:],
                initial=carry_use[:, c : c + 1],
                op0=ALU.add, op1=ALU.bypass,
            )
            kvcum = scrB16.tile([P, TL_], bf16, tag="b16")
            nc.vector.tensor_tensor_scan(
                out=kvcum[:], data0=kvl[:], data1=kvl[:],
                initial=carry_use[:, ND + c : ND + c + 1],
                op0=ALU.add, op1=ALU.bypass,
            )
            lw = scrB32.tile([P, TL_], f32, tag="b32")
            nc.scalar.activation(lw[:], wcum[:], AF.Ln, bias=eps6_b[:])
            rw = scrB16.tile([P, TL_], bf16, tag="b16")
            nc.scalar.activation(rw[:], lw[:], AF.Exp, scale=-1.0)
            yc = scrB16.tile([P, TL_], bf16, tag="b16")
            nc.vector.tensor_mul(out=yc[:], in0=kvcum[:], in1=rw[:])
            nc.gpsimd.dma_start(out=y_sp[c], in_=yc[:])

        # ---- phase 4bB: out = sigmoid(q * inv_q) * y ----
        # All-bf16 so every DVE op hits 2x mode; vector-engine only (the
        # gpsimd tensor ops are ~5x slower); bf16 output DMA (host widens).
        for c in range(ND):
            qc = scrB16.tile([P, TL_], bf16, tag="b16")
            nc.sync.dma_start(out=qc[:], in_=q_sp[c])
            qi = scrB16.tile([P, TL_], bf16, tag="b16")
            nc.vector.tensor_mul(out=qi[:], in0=qc[:], in1=inv_q[:])
            sg = scrB16.tile([P, TL_], bf16, tag="b16")
            nc.scalar.activation(sg[:], qi[:], AF.Sigmoid)
            yl = scrB16.tile([P, TL_], bf16, tag="b16")
            nc.sync.dma_start(out=yl[:], in_=y_sp[c])
            outc = scrB16.tile([P, TL_], bf16, tag="b16")
            nc.vector.tensor_mul(out=outc[:], in0=sg[:], in1=yl[:])
            nc.gpsimd.dma_start(out=out_h[c], in_=outc[:])

    nc.finalize()
    return nc


def make_in_maps(x, w_qkv, D_=D, TL_=TL, n_cores=NCORES):
    """Host-side shard + layout prep. Returns per-core input dicts."""
    P = 128
    ND = D_ // P
    E = w_qkv.shape[0]
    n_eblk = E // P
    b_count = x.shape[0]
    halves = n_cores // b_count

    # wT tiled: [e_blk, p, do, pe] with wtile[blk, p, do, e] = w_qkv[blk*128+e, do*128+p]
    wt = (
        np.ascontiguousarray(
            w_qkv.T.reshape(ND, P, n_eblk, P).transpose(2, 1, 0, 3)
        ).astype(BF16)
    )

    in_maps = []
    for core in range(n_cores):
        b, h = divmod(core, halves)
        shard = x[b, h * TL_ : (h + 1) * TL_, :]  # [TL, D]
        xt = np.ascontiguousarray(shard.T.reshape(ND, P, TL_).transpose(1, 0, 2)).astype(
            BF16
        )
        odd = float(h % 2 == 1)
        in_maps.append(
            {
                "xT": xt,
                "wT": wt,
                "cmask": np.full((P, 1), odd, dtype=np.float32),
                "smask": np.full((P, 1), 1.0 - odd, dtype=np.float32),
            }
        )
    return in_maps


def assemble_output(results, x, D_=D, TL_=TL, n_cores=NCORES):
    b_count = x.shape[0]
    halves = n_cores // b_count
    out2 = np.empty((b_count, halves * TL_, D_), dtype=np.float32)
    for core in range(n_cores):
        b, h = divmod(core, halves)
        outT = results[core]["outT"].reshape(D_, TL_)  # [d, t] bf16
        out2[b, h * TL_ : (h + 1) * TL_, :] = outT.T.astype(np.float32)
    return out2


_CACHED_NC = None


def kernel(x, w_qkv):
    global _CACHED_NC
    from concourse.bass_utils import run_bass_kernel_spmd

    x = np.asarray(x, dtype=np.float32)
    w_qkv = np.asarray(w_qkv, dtype=np.float32)

    if _CACHED_NC is None:
        _CACHED_NC = build_kernel()
    in_maps = make_in_maps(x, w_qkv)
    res = run_bass_kernel_spmd(_CACHED_NC, in_maps, core_ids=list(range(NCORES)))
    out2 = assemble_output(res.results, x)
    return (x, out2)



# revision 17
# speedup vs baseline: 1.0832x; 1.0832x over previous
"""AttentionFreeTransformer kernel for 8 TRN2 NeuronCores.

Reference computation (B=4, T=4096, D=2048):
    qkv = rmsnorm(x) @ w_qkv.T            # [B, T, 3D]
    q, k, v = split(qkv)
    q = rmsnorm(q); k = rmsnorm(k)
    w = exp(k); kv = w * v
    y = cumsum(kv, T) / (cumsum(w, T) + 1e-6)
    out = (x, sigmoid(q) * y)

Sharding: core = 2*b + h owns batch b, sequence half h (TL=2048 tokens).
All device tensors live in transposed layout [channel partitions, token free]
so the T-cumsum is a DVE tensor_tensor_scan along the free axis and the
cross-core carry (first-half column sums -> second-half core) is the scan's
per-partition `initial`, exchanged with one 16KB pairwise AllReduce.

Algebraic notes:
  - rmsnorm(x)'s per-token scale inv_x factors out of the projection:
    qkv_ref = inv_x[t] * (x @ w.T).  q and k are re-rmsnormed, which cancels
    the inv_x factor (up to eps), so only v needs inv_x applied.
  - rsqrt and reciprocal are computed as exp(-0.5*ln(.)) / exp(-ln(.)) on the
    scalar engine (natural_log_exp table set; Rsqrt/Reciprocal ACT funcs are
    banned for accuracy, DVE reciprocal is 8 cyc/elem).
"""

import sys

sys.path.insert(0, "/opt/trn_rl_repo")

import numpy as np
import ml_dtypes

import concourse.bass as bass
import concourse.bacc as bacc_mod
import concourse.mybir as mybir
from concourse.bass import ds, ts
from concourse.tile import TileContext

BF16 = ml_dtypes.bfloat16
F32EPS = float(np.finfo(np.float32).eps)

B, T, D = 4, 4096, 2048
NCORES = 8
TL = T // 2  # tokens per core

AF = mybir.ActivationFunctionType
ALU = mybir.AluOpType


class _Bacc(bacc_mod.Bacc):
    """Bacc whose act-table chooser maps all our funcs to one set.

    The default chooser assigns each activation to the first table set
    containing its func, which alternates exp_and_others / natural_log for
    interleaved Exp/Ln and costs ~2.7us per ACT_TABLE_LOAD. Filtering the
    candidate funcs (set indices preserved -- walrus maps id -> its own
    act_info.json) forces Exp/Ln/Square/Copy -> natural_log_exp_and_others
    and Sigmoid -> sigmoid_and_others: 2 loads total."""

    def insert_act_table_loads(self):
        from concourse.hw_specs import get_activation_tables
        from concourse.bacc import _bass_rust

        has_activation = any(
            isinstance(i, mybir.InstActivation)
            for b in self.main_func.blocks
            for i in b.instructions
        )
        if not has_activation:
            return
        ours = {AF.Exp, AF.Ln, AF.Square, AF.Copy, AF.Identity, AF.Sigmoid}
        tables = []
        for name, funcs in get_activation_tables(self.m.arch).items():
            if name == "natural_log_exp_and_others":
                tables.append((name, funcs))
            elif name == "sigmoid_and_others":
                tables.append((name, (funcs - ours) | {AF.Sigmoid}))
            else:
                tables.append((name, funcs - ours))
        _bass_rust.insert_act_table_loads(self, tables)


def build_kernel(D_=D, TL_=TL, n_cores=NCORES):
    P = 128
    ND = D_ // P          # channel subtiles (16)
    NT = TL_ // 512       # 512-token psum chunks (4)
    E_BLKS = 3 * ND       # e tiles across q|k|v
    inv_scale = 1.0 / D_

    nc = _Bacc(target_bir_lowering=False, num_devices=n_cores)

    f32 = mybir.dt.float32
    bf16 = mybir.dt.bfloat16

    xT_h = nc.declare_dram_parameter("xT", [P, ND, TL_], bf16, isOutput=False)
    wT_h = nc.declare_dram_parameter("wT", [E_BLKS, P, ND, P], bf16, isOutput=False)
    cmask_h = nc.declare_dram_parameter("cmask", [P, 1], f32, isOutput=False)
    smask_h = nc.declare_dram_parameter("smask", [P, 1], f32, isOutput=False)
    out_h = nc.declare_dram_parameter("outT", [ND, P, TL_], bf16, isOutput=True)

    ones_col_h = nc.inline_tensor(np.ones((P, 1), dtype=BF16), name="ones_col")
    ones_row_h = nc.inline_tensor(np.ones((1, P), dtype=BF16), name="ones_row")

    groups = [[i, i + 1] for i in range(0, n_cores, 2)]

    with (
        TileContext(nc) as tc,
        tc.tile_pool(name="const", bufs=1) as const,
        tc.tile_pool(name="wstream", bufs=2) as wstream,
        tc.tile_pool(name="scr32", bufs=2) as scr32,
        tc.tile_pool(name="scr16", bufs=4) as scr16,
        tc.tile_pool(name="scrB16", bufs=6) as scrB16,
        tc.tile_pool(name="scrB32", bufs=2) as scrB32,
        tc.tile_pool(name="ytiles", bufs=ND) as ytiles,
        tc.tile_pool(name="invp", bufs=2) as invp,
        tc.tile_pool(name="rows", bufs=1) as rows,
        tc.tile_pool(name="mmps", bufs=6, space="PSUM") as mmps,
        tc.tile_pool(name="ssqps", bufs=2, space="PSUM") as ssqps,
        tc.tile_pool(name="spill", bufs=1, space="DRAM") as spill,
    ):
        # ---- resident tiles ----
        xT_sb = const.tile([P, ND, TL_], bf16, tag="xT_sb")
        for do in range(ND):
            nc.sync.dma_start(out=xT_sb[:, do, :], in_=xT_h[:, do, :])

        ones_col = const.tile([P, 1], bf16, tag="ones_col")
        nc.sync.dma_start(out=ones_col[:], in_=ones_col_h[:])
        ones_row = const.tile([1, P], bf16, tag="ones_row")
        nc.sync.dma_start(out=ones_row[:], in_=ones_row_h[:])
        cmask = const.tile([P, 1], f32, tag="cmask")
        nc.sync.dma_start(out=cmask[:], in_=cmask_h[:])
        smask = const.tile([P, 1], f32, tag="smask")
        nc.sync.dma_start(out=smask[:], in_=smask_h[:])

        eps_b = const.tile([P, 1], f32, tag="eps_b")
        nc.vector.memset(eps_b[:], F32EPS)
        eps6_b = const.tile([P, 1], f32, tag="eps6_b")
        nc.vector.memset(eps6_b[:], 1e-6)

        carry_both = const.tile([P, 2 * ND], f32, tag="carry_both")
        carry_use = const.tile([P, 2 * ND], f32, tag="carry_use")

        # ---- DRAM spill arrays ----
        q_sp = spill.tile([ND, P, TL_], bf16, tag="q_sp")
        k_sp = spill.tile([ND, P, TL_], bf16, tag="k_sp")
        w_sp = spill.tile([ND, P, TL_], bf16, tag="w_sp")
        kv_sp = spill.tile([ND, P, TL_], bf16, tag="kv_sp")
        cc_in = spill.tile([P, 2 * ND], f32, tag="cc_in")
        cc_out = spill.tile([P, 2 * ND], f32, tag="cc_out")

        def inv_chain(ssq_tiles, name, extra_scale):
            """returns inv[p, t] = (ssq[t]/D + eps) ** extra_scale (replicated).

            ssq_tiles: NT psum tiles [1, 512] holding per-token sums of squares.
            """
            row = rows.tile([1, TL_], bf16, tag="row")
            for tci in range(NT):
                bank, rp = divmod(tci, 2)
                nc.scalar.copy(
                    out=row[:, ts(tci, 512)],
                    in_=ssq_tiles[bank][32 * rp : 32 * rp + 1, :],
                )
            lnv = scrB32.tile([P, TL_], f32, tag="b32")
            for tci in range(NT):
                rep = ssqps.tile([P, 512], f32, tag="ssq", name="rep")
                nc.tensor.matmul(
                    out=rep[:],
                    lhsT=ones_row[:],
                    rhs=row[:, ts(tci, 512)],
                    start=True,
                    stop=True,
                )
                nc.scalar.activation(
                    lnv[:, ts(tci, 512)], rep[:], AF.Ln,
                    bias=eps_b[:], scale=inv_scale,
                )
            dest = invp.tile([P, TL_], bf16, tag="inv", name=name)
            nc.scalar.activation(dest[:], lnv[:], AF.Exp, scale=extra_scale)
            return dest

        def projection(e_blk_base, j, psum_tiles):
            """One 128-wide output-channel tile of qkv = wT.T @ xT."""
            wsb = wstream.tile([P, ND, P], bf16, tag="wsb")
            nc.sync.dma_start(out=wsb[:], in_=wT_h[e_blk_base + j])
            for tci in range(NT):
                for do in range(ND):
                    nc.tensor.matmul(
                        out=psum_tiles[tci][:],
                        lhsT=wsb[:, do, :],
                        rhs=xT_sb[:, do, ts(tci, 512)],
                        start=(do == 0),
                        stop=(do == ND - 1),
                    )

        def ssq_accumulate(src_sb, ssq_tiles, j):
            """Accumulate per-token sum of squares of src_sb into ssq psum."""
            sq = scr16.tile([P, TL_], bf16, tag="s16")
            nc.scalar.activation(sq[:], src_sb[:], AF.Square)
            for tci in range(NT):
                bank, rp = divmod(tci, 2)
                nc.tensor.matmul(
                    out=ssq_tiles[bank][32 * rp : 32 * rp + 1, :],
                    lhsT=ones_col[:],
                    rhs=sq[:, ts(tci, 512)],
                    start=(j == 0),
                    stop=(j == ND - 1),
                )

        # ---- phase K: k projection, spill, ssq (runs first so PE starts hot) ----
        kssq = [ssqps.tile([64, 512], f32, tag="ssq", name=f"kssq{i}") for i in range((NT + 1) // 2)]
        for j in range(ND):
            pk = [mmps.tile([P, 512], f32, tag="mm", name=f"pk{j}_{i}") for i in range(NT)]
            projection(ND, j, pk)
            ksb = scr16.tile([P, TL_], bf16, tag="s16")
            for tci in range(NT):
                nc.scalar.copy(out=ksb[:, ts(tci, 512)], in_=pk[tci][:])
            nc.gpsimd.dma_start(out=k_sp[j], in_=ksb[:])
            ssq_accumulate(ksb, kssq, j)
        inv_k = inv_chain(kssq, "inv_k", -0.5)

        # ---- phase X: sum of squares of x, inv_x (hidden under phase K) ----
        xssq = [ssqps.tile([64, 512], f32, tag="ssq", name=f"xssq{i}") for i in range((NT + 1) // 2)]
        for do in range(ND):
            sq = scr16.tile([P, TL_], bf16, tag="s16")
            nc.scalar.activation(sq[:], xT_sb[:, do, :], AF.Square)
            for tci in range(NT):
                bank, rp = divmod(tci, 2)
                nc.tensor.matmul(
                    out=xssq[bank][32 * rp : 32 * rp + 1, :],
                    lhsT=ones_col[:],
                    rhs=sq[:, ts(tci, 512)],
                    start=(do == 0),
                    stop=(do == ND - 1),
                )
        inv_x = inv_chain(xssq, "inv_x", -0.5)

        # ---- phase V + 4a: v projection, w=exp(k*invk), kv, totals ----
        for c in range(ND):
            pv = [mmps.tile([P, 512], f32, tag="mm", name=f"pv{c}_{i}") for i in range(NT)]
            projection(2 * ND, c, pv)
            vsb = scrB16.tile([P, TL_], bf16, tag="b16")
            for tci in range(NT):
                nc.vector.tensor_mul(
                    out=vsb[:, ts(tci, 512)], in0=pv[tci][:],
                    in1=inv_x[:, ts(tci, 512)],
                )
            kc = scr16.tile([P, TL_], bf16, tag="s16")
            nc.sync.dma_start(out=kc[:], in_=k_sp[c])
            kn = scrB16.tile([P, TL_], bf16, tag="b16")
            nc.vector.tensor_mul(out=kn[:], in0=kc[:], in1=inv_k[:])
            wc = scr16.tile([P, TL_], bf16, tag="s16")
            nc.scalar.activation(
                wc[:], kn[:], AF.Exp, accum_out=carry_both[:, c : c + 1]
            )
            kvc = scr16.tile([P, TL_], bf16, tag="s16")
            nc.vector.scalar_tensor_tensor(
                out=kvc[:], in0=wc[:], scalar=1.0, in1=vsb[:],
                op0=ALU.mult, op1=ALU.mult,
                accum_out=carry_both[:, ND + c : ND + c + 1],
            )
            nc.gpsimd.dma_start(out=w_sp[c], in_=wc[:])
            nc.gpsimd.dma_start(out=kv_sp[c], in_=kvc[:])

        # ---- carry exchange send: even core's totals -> pairwise AllReduce ----
        snd = scr32.tile([P, 2 * ND], f32, tag="snd")
        nc.vector.tensor_scalar_mul(snd[:], carry_both[:], smask[:])
        nc.gpsimd.dma_start(out=cc_in[:], in_=snd[:])
        nc.gpsimd.collective_compute(
            "AllReduce",
            ALU.add,
            replica_groups=groups,
            ins=[cc_in[:]],
            outs=[cc_out[:]],
        )

        # ---- phase Q: q projection + interleaved scans ----
        # The collective-result receive is issued AFTER j==2's weight loads so
        # it cannot head-of-line-block the Sync DMA queue that feeds the
        # projection's wT stream (the baseline stalled the PE ~23us here).
        # The scan blocks are interleaved into the j-loop so their spill
        # reloads interleave with the weight loads in Sync-queue order and
        # their Ln/Exp spread evenly through ACT's program.
        y_tiles = [None] * ND

        def scan_block(c):
            wcl = scrB16.tile([P, TL_], bf16, tag="b16")
            nc.sync.dma_start(out=wcl[:], in_=w_sp[c])
            kvl = scrB16.tile([P, TL_], bf16, tag="b16")
            nc.sync.dma_start(out=kvl[:], in_=kv_sp[c])
            wcum = scrB16.tile([P, TL_], bf16, tag="b16")
            nc.vector.tensor_tensor_scan(
                out=wcum[:], data0=wcl[:], data1=wcl[:],
                initial=carry_use[:, c : c + 1],
                op0=ALU.add, op1=ALU.bypass,
            )
            kvcum = scrB16.tile([P, TL_], bf16, tag="b16")
            nc.vector.tensor_tensor_scan(
                out=kvcum[:], data0=kvl[:], data1=kvl[:],
                initial=carry_use[:, ND + c : ND + c + 1],
                op0=ALU.add, op1=ALU.bypass,
            )
            lw = scrB32.tile([P, TL_], f32, tag="b32")
            nc.scalar.activation(lw[:], wcum[:], AF.Ln, bias=eps6_b[:])
            rw = scrB16.tile([P, TL_], bf16, tag="b16")
            nc.scalar.activation(rw[:], lw[:], AF.Exp, scale=-1.0)
            yc = ytiles.tile([P, TL_], bf16, tag="y", name=f"y{c}")
            nc.vector.tensor_mul(out=yc[:], in0=kvcum[:], in1=rw[:])
            y_tiles[c] = yc

        scan_lead = min(3, ND - 1)
        qssq = [ssqps.tile([64, 512], f32, tag="ssq", name=f"qssq{i}") for i in range((NT + 1) // 2)]
        for j in range(ND):
            pq = [mmps.tile([P, 512], f32, tag="mm", name=f"pq{j}_{i}") for i in range(NT)]
            projection(0, j, pq)
            qsb = scr16.tile([P, TL_], bf16, tag="s16")
            for tci in range(NT):
                nc.scalar.copy(out=qsb[:, ts(tci, 512)], in_=pq[tci][:])
            nc.gpsimd.dma_start(out=q_sp[j], in_=qsb[:])
            ssq_accumulate(qsb, qssq, j)
            if j == min(2, ND - 1):
                rcv = scr32.tile([P, 2 * ND], f32, tag="snd")
                nc.sync.dma_start(out=rcv[:], in_=cc_out[:])
                nc.vector.tensor_scalar_mul(carry_use[:], rcv[:], cmask[:])
            if j >= scan_lead:
                scan_block(j - scan_lead)
        for c in range(ND - scan_lead, ND):
            scan_block(c)
        inv_q = inv_chain(qssq, "inv_q", -0.5)

        # ---- phase 4bB: out = sigmoid(q * inv_q) * y ----
        # All-bf16 so every DVE op hits 2x mode; vector-engine only (the
        # gpsimd tensor ops are ~5x slower); y stays resident in SBUF; bf16
        # output DMA (host widens).
        for c in range(ND):
            qc = scrB16.tile([P, TL_], bf16, tag="b16")
            nc.sync.dma_start(out=qc[:], in_=q_sp[c])
            qi = scrB16.tile([P, TL_], bf16, tag="b16")
            nc.vector.tensor_mul(out=qi[:], in0=qc[:], in1=inv_q[:])
            sg = scrB16.tile([P, TL_], bf16, tag="b16")
            nc.scalar.activation(sg[:], qi[:], AF.Sigmoid)
            outc = scrB16.tile([P, TL_], bf16, tag="b16")
            nc.vector.tensor_mul(out=outc[:], in0=sg[:], in1=y_tiles[c][:])
            nc.gpsimd.dma_start(out=out_h[c], in_=outc[:])

    nc.finalize()
    return nc


def make_in_maps(x, w_qkv, D_=D, TL_=TL, n_cores=NCORES):
    """Host-side shard + layout prep. Returns per-core input dicts."""
    P = 128
    ND = D_ // P
    E = w_qkv.shape[0]
    n_eblk = E // P
    b_count = x.shape[0]
    halves = n_cores // b_count

    # wT tiled: [e_blk, p, do, pe] with wtile[blk, p, do, e] = w_qkv[blk*128+e, do*128+p]
    wt = (
        np.ascontiguousarray(
            w_qkv.T.reshape(ND, P, n_eblk, P).transpose(2, 1, 0, 3)
        ).astype(BF16)
    )

    in_maps = []
    for core in range(n_cores):
        b, h = divmod(core, halves)
        shard = x[b, h * TL_ : (h + 1) * TL_, :]  # [TL, D]
        xt = np.ascontiguousarray(shard.T.reshape(ND, P, TL_).transpose(1, 0, 2)).astype(
            BF16
        )
        odd = float(h % 2 == 1)
        in_maps.append(
            {
                "xT": xt,
                "wT": wt,
                "cmask": np.full((P, 1), odd, dtype=np.float32),
                "smask": np.full((P, 1), 1.0 - odd, dtype=np.float32),
            }
        )
    return in_maps


def assemble_output(results, x, D_=D, TL_=TL, n_cores=NCORES):
    b_count = x.shape[0]
    halves = n_cores // b_count
    out2 = np.empty((b_count, halves * TL_, D_), dtype=np.float32)
    for core in range(n_cores):
        b, h = divmod(core, halves)
        outT = results[core]["outT"].reshape(D_, TL_)  # [d, t] bf16
        out2[b, h * TL_ : (h + 1) * TL_, :] = outT.T.astype(np.float32)
    return out2


_CACHED_NC = None


def kernel(x, w_qkv):
    global _CACHED_NC
    from concourse.bass_utils import run_bass_kernel_spmd

    x = np.asarray(x, dtype=np.float32)
    w_qkv = np.asarray(w_qkv, dtype=np.float32)

    if _CACHED_NC is None:
        _CACHED_NC = build_kernel()
    in_maps = make_in_maps(x, w_qkv)
    res = run_bass_kernel_spmd(_CACHED_NC, in_maps, core_ids=list(range(NCORES)))
    out2 = assemble_output(res.results, x)
    return (x, out2)



# revision 21
# speedup vs baseline: 1.1018x; 1.0172x over previous
"""AttentionFreeTransformer kernel for 8 TRN2 NeuronCores.

Reference computation (B=4, T=4096, D=2048):
    qkv = rmsnorm(x) @ w_qkv.T            # [B, T, 3D]
    q, k, v = split(qkv)
    q = rmsnorm(q); k = rmsnorm(k)
    w = exp(k); kv = w * v
    y = cumsum(kv, T) / (cumsum(w, T) + 1e-6)
    out = (x, sigmoid(q) * y)

Sharding: core = 2*b + h owns batch b, sequence half h (TL=2048 tokens).
All device tensors live in transposed layout [channel partitions, token free]
so the T-cumsum is a DVE tensor_tensor_scan along the free axis and the
cross-core carry (first-half column sums -> second-half core) is the scan's
per-partition `initial`, exchanged with one 16KB pairwise AllReduce.

Algebraic notes:
  - rmsnorm(x)'s per-token scale inv_x factors out of the projection:
    qkv_ref = inv_x[t] * (x @ w.T).  q and k are re-rmsnormed, which cancels
    the inv_x factor (up to eps), so only v needs inv_x applied.
  - rsqrt and reciprocal are computed as exp(-0.5*ln(.)) / exp(-ln(.)) on the
    scalar engine (natural_log_exp table set; Rsqrt/Reciprocal ACT funcs are
    banned for accuracy, DVE reciprocal is 8 cyc/elem).
"""

import sys

sys.path.insert(0, "/opt/trn_rl_repo")

import numpy as np
import ml_dtypes

import concourse.bass as bass
import concourse.bacc as bacc_mod
import concourse.mybir as mybir
from concourse.bass import ds, ts
from concourse.tile import TileContext

BF16 = ml_dtypes.bfloat16
F32EPS = float(np.finfo(np.float32).eps)

B, T, D = 4, 4096, 2048
NCORES = 8
TL = T // 2  # tokens per core

AF = mybir.ActivationFunctionType
ALU = mybir.AluOpType


class _Bacc(bacc_mod.Bacc):
    """Bacc whose act-table chooser maps all our funcs to one set.

    The default chooser assigns each activation to the first table set
    containing its func, which alternates exp_and_others / natural_log for
    interleaved Exp/Ln and costs ~2.7us per ACT_TABLE_LOAD. Filtering the
    candidate funcs (set indices preserved -- walrus maps id -> its own
    act_info.json) forces Exp/Ln/Square/Copy -> natural_log_exp_and_others
    and Sigmoid -> sigmoid_and_others: 2 loads total."""

    def insert_act_table_loads(self):
        from concourse.hw_specs import get_activation_tables
        from concourse.bacc import _bass_rust

        has_activation = any(
            isinstance(i, mybir.InstActivation)
            for b in self.main_func.blocks
            for i in b.instructions
        )
        if not has_activation:
            return
        ours = {AF.Exp, AF.Ln, AF.Square, AF.Copy, AF.Identity, AF.Sigmoid}
        tables = []
        for name, funcs in get_activation_tables(self.m.arch).items():
            if name == "natural_log_exp_and_others":
                tables.append((name, funcs))
            elif name == "sigmoid_and_others":
                tables.append((name, (funcs - ours) | {AF.Sigmoid}))
            else:
                tables.append((name, funcs - ours))
        _bass_rust.insert_act_table_loads(self, tables)


def build_kernel(D_=D, TL_=TL, n_cores=NCORES):
    P = 128
    ND = D_ // P          # channel subtiles (16)
    NT = TL_ // 512       # 512-token psum chunks (4)
    E_BLKS = 3 * ND       # e tiles across q|k|v
    inv_scale = 1.0 / D_

    nc = _Bacc(target_bir_lowering=False, num_devices=n_cores)

    f32 = mybir.dt.float32
    bf16 = mybir.dt.bfloat16

    xT_h = nc.declare_dram_parameter("xT", [P, ND, TL_], bf16, isOutput=False)
    wT_h = nc.declare_dram_parameter("wT", [E_BLKS, P, ND, P], bf16, isOutput=False)
    cmask_h = nc.declare_dram_parameter("cmask", [P, 1], f32, isOutput=False)
    smask_h = nc.declare_dram_parameter("smask", [P, 1], f32, isOutput=False)
    out_h = nc.declare_dram_parameter("outT", [ND, P, TL_], bf16, isOutput=True)

    ones_col_h = nc.inline_tensor(np.ones((P, 1), dtype=BF16), name="ones_col")
    ones_row_h = nc.inline_tensor(np.ones((1, P), dtype=BF16), name="ones_row")

    groups = [[i, i + 1] for i in range(0, n_cores, 2)]

    with (
        TileContext(nc) as tc,
        tc.tile_pool(name="const", bufs=1) as const,
        tc.tile_pool(name="wstream", bufs=2) as wstream,
        tc.tile_pool(name="scr32", bufs=2) as scr32,
        tc.tile_pool(name="scr16", bufs=4) as scr16,
        tc.tile_pool(name="scrB16", bufs=6) as scrB16,
        tc.tile_pool(name="scrB32", bufs=2) as scrB32,
        tc.tile_pool(name="ytiles", bufs=ND) as ytiles,
        tc.tile_pool(name="invp", bufs=2) as invp,
        tc.tile_pool(name="rows", bufs=1) as rows,
        tc.tile_pool(name="mmps", bufs=6, space="PSUM") as mmps,
        tc.tile_pool(name="ssqps", bufs=2, space="PSUM") as ssqps,
        tc.tile_pool(name="spill", bufs=1, space="DRAM") as spill,
    ):
        # ---- resident tiles ----
        # The first two weight tiles are DMA'd ahead of the xT stream so the
        # PE's first matmuls start ~3us in and chase the arriving xT slices.
        wsb_pre = [None, None]
        wsb_pre[0] = wstream.tile([P, ND, P], bf16, tag="wsb", name="wsb_pre0")
        nc.sync.dma_start(out=wsb_pre[0][:], in_=wT_h[ND + 0])
        if ND > 1:
            wsb_pre[1] = wstream.tile([P, ND, P], bf16, tag="wsb", name="wsb_pre1")
            nc.sync.dma_start(out=wsb_pre[1][:], in_=wT_h[ND + 1])

        xT_sb = const.tile([P, ND, TL_], bf16, tag="xT_sb")
        for do in range(ND):
            nc.sync.dma_start(out=xT_sb[:, do, :], in_=xT_h[:, do, :])

        ones_col = const.tile([P, 1], bf16, tag="ones_col")
        nc.sync.dma_start(out=ones_col[:], in_=ones_col_h[:])
        ones_row = const.tile([1, P], bf16, tag="ones_row")
        nc.sync.dma_start(out=ones_row[:], in_=ones_row_h[:])
        cmask = const.tile([P, 1], f32, tag="cmask")
        nc.sync.dma_start(out=cmask[:], in_=cmask_h[:])
        smask = const.tile([P, 1], f32, tag="smask")
        nc.sync.dma_start(out=smask[:], in_=smask_h[:])

        eps_b = const.tile([P, 1], f32, tag="eps_b")
        nc.vector.memset(eps_b[:], F32EPS)
        eps6_b = const.tile([P, 1], f32, tag="eps6_b")
        nc.vector.memset(eps6_b[:], 1e-6)

        carry_both = const.tile([P, 2 * ND], f32, tag="carry_both")
        carry_use = const.tile([P, 2 * ND], f32, tag="carry_use")

        # ---- DRAM spill arrays ----
        q_sp = spill.tile([ND, P, TL_], bf16, tag="q_sp")
        k_sp = spill.tile([ND, P, TL_], bf16, tag="k_sp")
        w_sp = spill.tile([ND, P, TL_], bf16, tag="w_sp")
        kv_sp = spill.tile([ND, P, TL_], bf16, tag="kv_sp")
        cc_in = spill.tile([P, 2 * ND], f32, tag="cc_in")
        cc_out = spill.tile([P, 2 * ND], f32, tag="cc_out")

        def inv_chain(ssq_tiles, name, extra_scale):
            """returns inv[p, t] = (ssq[t]/D + eps) ** extra_scale (replicated).

            ssq_tiles: NT psum tiles [1, 512] holding per-token sums of squares.
            """
            row = rows.tile([1, TL_], bf16, tag="row")
            for tci in range(NT):
                bank, rp = divmod(tci, 2)
                nc.scalar.copy(
                    out=row[:, ts(tci, 512)],
                    in_=ssq_tiles[bank][32 * rp : 32 * rp + 1, :],
                )
            lnv = scrB32.tile([P, TL_], f32, tag="b32")
            for tci in range(NT):
                rep = ssqps.tile([P, 512], f32, tag="ssq", name="rep")
                nc.tensor.matmul(
                    out=rep[:],
                    lhsT=ones_row[:],
                    rhs=row[:, ts(tci, 512)],
                    start=True,
                    stop=True,
                )
                nc.scalar.activation(
                    lnv[:, ts(tci, 512)], rep[:], AF.Ln,
                    bias=eps_b[:], scale=inv_scale,
                )
            dest = invp.tile([P, TL_], bf16, tag="inv", name=name)
            nc.scalar.activation(dest[:], lnv[:], AF.Exp, scale=extra_scale)
            return dest

        def wload(e_blk_base, j):
            wsb = wstream.tile([P, ND, P], bf16, tag="wsb")
            nc.sync.dma_start(out=wsb[:], in_=wT_h[e_blk_base + j])
            return wsb

        def projection(e_blk_base, j, psum_tiles, wsb=None):
            """One 128-wide output-channel tile of qkv = wT.T @ xT.

            do-outer order: each xT contraction slice is consumed in one
            burst of NT matmuls, so the first j-iteration streams behind the
            initial xT DMAs instead of waiting for all of them.
            """
            if wsb is None:
                wsb = wload(e_blk_base, j)
            for do in range(ND):
                for tci in range(NT):
                    nc.tensor.matmul(
                        out=psum_tiles[tci][:],
                        lhsT=wsb[:, do, :],
                        rhs=xT_sb[:, do, ts(tci, 512)],
                        start=(do == 0),
                        stop=(do == ND - 1),
                    )

        def ssq_accumulate(src_sb, ssq_tiles, j):
            """Accumulate per-token sum of squares of src_sb into ssq psum."""
            sq = scr16.tile([P, TL_], bf16, tag="s16")
            nc.scalar.activation(sq[:], src_sb[:], AF.Square)
            for tci in range(NT):
                bank, rp = divmod(tci, 2)
                nc.tensor.matmul(
                    out=ssq_tiles[bank][32 * rp : 32 * rp + 1, :],
                    lhsT=ones_col[:],
                    rhs=sq[:, ts(tci, 512)],
                    start=(j == 0),
                    stop=(j == ND - 1),
                )

        # ---- phase K: k projection, spill, ssq (runs first so PE starts hot) ----
        kssq = [ssqps.tile([64, 512], f32, tag="ssq", name=f"kssq{i}") for i in range((NT + 1) // 2)]
        for j in range(ND):
            pk = [mmps.tile([P, 512], f32, tag="mm", name=f"pk{j}_{i}") for i in range(NT)]
            projection(ND, j, pk, wsb=wsb_pre[j] if j < 2 else None)
            ksb = scr16.tile([P, TL_], bf16, tag="s16")
            for tci in range(NT):
                nc.scalar.copy(out=ksb[:, ts(tci, 512)], in_=pk[tci][:])
            nc.gpsimd.dma_start(out=k_sp[j], in_=ksb[:])
            ssq_accumulate(ksb, kssq, j)
        inv_k = inv_chain(kssq, "inv_k", -0.5)

        # ---- phase X: sum of squares of x, inv_x (hidden under phase K) ----
        xssq = [ssqps.tile([64, 512], f32, tag="ssq", name=f"xssq{i}") for i in range((NT + 1) // 2)]
        for do in range(ND):
            sq = scr16.tile([P, TL_], bf16, tag="s16")
            nc.scalar.activation(sq[:], xT_sb[:, do, :], AF.Square)
            for tci in range(NT):
                bank, rp = divmod(tci, 2)
                nc.tensor.matmul(
                    out=xssq[bank][32 * rp : 32 * rp + 1, :],
                    lhsT=ones_col[:],
                    rhs=sq[:, ts(tci, 512)],
                    start=(do == 0),
                    stop=(do == ND - 1),
                )
        inv_x = inv_chain(xssq, "inv_x", -0.5)

        # ---- phase V + 4a: v projection, w=exp(k*invk), kv, totals ----
        for c in range(ND):
            pv = [mmps.tile([P, 512], f32, tag="mm", name=f"pv{c}_{i}") for i in range(NT)]
            projection(2 * ND, c, pv)
            vsb = scrB16.tile([P, TL_], bf16, tag="b16")
            for tci in range(NT):
                nc.vector.tensor_mul(
                    out=vsb[:, ts(tci, 512)], in0=pv[tci][:],
                    in1=inv_x[:, ts(tci, 512)],
                )
            kc = scr16.tile([P, TL_], bf16, tag="s16")
            nc.sync.dma_start(out=kc[:], in_=k_sp[c])
            kn = scrB16.tile([P, TL_], bf16, tag="b16")
            nc.vector.tensor_mul(out=kn[:], in0=kc[:], in1=inv_k[:])
            wc = scr16.tile([P, TL_], bf16, tag="s16")
            nc.scalar.activation(
                wc[:], kn[:], AF.Exp, accum_out=carry_both[:, c : c + 1]
            )
            kvc = scr16.tile([P, TL_], bf16, tag="s16")
            nc.vector.scalar_tensor_tensor(
                out=kvc[:], in0=wc[:], scalar=1.0, in1=vsb[:],
                op0=ALU.mult, op1=ALU.mult,
                accum_out=carry_both[:, ND + c : ND + c + 1],
            )
            nc.gpsimd.dma_start(out=w_sp[c], in_=wc[:])
            nc.gpsimd.dma_start(out=kv_sp[c], in_=kvc[:])

        # ---- carry exchange send: even core's totals -> pairwise AllReduce ----
        snd = scr32.tile([P, 2 * ND], f32, tag="snd")
        nc.vector.tensor_scalar_mul(snd[:], carry_both[:], smask[:])
        nc.gpsimd.dma_start(out=cc_in[:], in_=snd[:])
        nc.gpsimd.collective_compute(
            "AllReduce",
            ALU.add,
            replica_groups=groups,
            ins=[cc_in[:]],
            outs=[cc_out[:]],
        )

        # ---- phase Q: q projection + interleaved scans ----
        # The collective-result receive is issued AFTER j==2's weight loads so
        # it cannot head-of-line-block the Sync DMA queue that feeds the
        # projection's wT stream (the baseline stalled the PE ~23us here).
        # The scan blocks are interleaved into the j-loop so their spill
        # reloads interleave with the weight loads in Sync-queue order and
        # their Ln/Exp spread evenly through ACT's program.
        y_tiles = [None] * ND

        def scan_block(c):
            wcl = scrB16.tile([P, TL_], bf16, tag="b16")
            nc.sync.dma_start(out=wcl[:], in_=w_sp[c])
            kvl = scrB16.tile([P, TL_], bf16, tag="b16")
            nc.sync.dma_start(out=kvl[:], in_=kv_sp[c])
            wcum = scrB16.tile([P, TL_], bf16, tag="b16")
            nc.vector.tensor_tensor_scan(
                out=wcum[:], data0=wcl[:], data1=wcl[:],
                initial=carry_use[:, c : c + 1],
                op0=ALU.add, op1=ALU.bypass,
            )
            kvcum = scrB16.tile([P, TL_], bf16, tag="b16")
            nc.vector.tensor_tensor_scan(
                out=kvcum[:], data0=kvl[:], data1=kvl[:],
                initial=carry_use[:, ND + c : ND + c + 1],
                op0=ALU.add, op1=ALU.bypass,
            )
            lw = scrB32.tile([P, TL_], f32, tag="b32")
            nc.scalar.activation(lw[:], wcum[:], AF.Ln, bias=eps6_b[:])
            rw = scrB16.tile([P, TL_], bf16, tag="b16")
            nc.scalar.activation(rw[:], lw[:], AF.Exp, scale=-1.0)
            yc = ytiles.tile([P, TL_], bf16, tag="y", name=f"y{c}")
            nc.vector.tensor_mul(out=yc[:], in0=kvcum[:], in1=rw[:])
            y_tiles[c] = yc

        scan_lead = min(3, ND - 1)
        qssq = [ssqps.tile([64, 512], f32, tag="ssq", name=f"qssq{i}") for i in range((NT + 1) // 2)]
        for j in range(ND):
            pq = [mmps.tile([P, 512], f32, tag="mm", name=f"pq{j}_{i}") for i in range(NT)]
            projection(0, j, pq)
            qsb = scr16.tile([P, TL_], bf16, tag="s16")
            for tci in range(NT):
                nc.scalar.copy(out=qsb[:, ts(tci, 512)], in_=pq[tci][:])
            nc.gpsimd.dma_start(out=q_sp[j], in_=qsb[:])
            ssq_accumulate(qsb, qssq, j)
            if j == min(2, ND - 1):
                rcv = scr32.tile([P, 2 * ND], f32, tag="snd")
                nc.sync.dma_start(out=rcv[:], in_=cc_out[:])
                nc.vector.tensor_scalar_mul(carry_use[:], rcv[:], cmask[:])
            if j >= scan_lead:
                scan_block(j - scan_lead)
        for c in range(ND - scan_lead, ND):
            scan_block(c)
        inv_q = inv_chain(qssq, "inv_q", -0.5)

        # ---- phase 4bB: out = sigmoid(q * inv_q) * y ----
        # All-bf16 so every DVE op hits 2x mode; vector-engine only (the
        # gpsimd tensor ops are ~5x slower); y stays resident in SBUF; bf16
        # output DMA (host widens).  Software-pipelined: qi for tile c+1 is
        # issued BEFORE outc for tile c so the DVE's in-order queue does not
        # serialize the sigmoid latency into the chain.
        def qi_block(c):
            qc = scrB16.tile([P, TL_], bf16, tag="b16")
            nc.sync.dma_start(out=qc[:], in_=q_sp[c])
            qi = scrB16.tile([P, TL_], bf16, tag="b16")
            nc.vector.tensor_mul(out=qi[:], in0=qc[:], in1=inv_q[:])
            return qi

        qi_cur = qi_block(0)
        for c in range(ND):
            sg = scrB16.tile([P, TL_], bf16, tag="b16")
            nc.scalar.activation(sg[:], qi_cur[:], AF.Sigmoid)
            if c + 1 < ND:
                qi_cur = qi_block(c + 1)
            outc = scrB16.tile([P, TL_], bf16, tag="b16")
            nc.vector.tensor_mul(out=outc[:], in0=sg[:], in1=y_tiles[c][:])
            nc.gpsimd.dma_start(out=out_h[c], in_=outc[:])

    nc.finalize()
    return nc


def make_in_maps(x, w_qkv, D_=D, TL_=TL, n_cores=NCORES):
    """Host-side shard + layout prep. Returns per-core input dicts."""
    P = 128
    ND = D_ // P
    E = w_qkv.shape[0]
    n_eblk = E // P
    b_count = x.shape[0]
    halves = n_cores // b_count

    # wT tiled: [e_blk, p, do, pe] with wtile[blk, p, do, e] = w_qkv[blk*128+e, do*128+p]
    wt = (
        np.ascontiguousarray(
            w_qkv.T.reshape(ND, P, n_eblk, P).transpose(2, 1, 0, 3)
        ).astype(BF16)
    )

    in_maps = []
    for core in range(n_cores):
        b, h = divmod(core, halves)
        shard = x[b, h * TL_ : (h + 1) * TL_, :]  # [TL, D]
        xt = np.ascontiguousarray(shard.T.reshape(ND, P, TL_).transpose(1, 0, 2)).astype(
            BF16
        )
        odd = float(h % 2 == 1)
        in_maps.append(
            {
                "xT": xt,
                "wT": wt,
                "cmask": np.full((P, 1), odd, dtype=np.float32),
                "smask": np.full((P, 1), 1.0 - odd, dtype=np.float32),
            }
        )
    return in_maps


def assemble_output(results, x, D_=D, TL_=TL, n_cores=NCORES):
    b_count = x.shape[0]
    halves = n_cores // b_count
    out2 = np.empty((b_count, halves * TL_, D_), dtype=np.float32)
    for core in range(n_cores):
        b, h = divmod(core, halves)
        outT = results[core]["outT"].reshape(D_, TL_)  # [d, t] bf16
        out2[b, h * TL_ : (h + 1) * TL_, :] = outT.T.astype(np.float32)
    return out2


_CACHED_NC = None


def kernel(x, w_qkv):
    global _CACHED_NC
    from concourse.bass_utils import run_bass_kernel_spmd

    x = np.asarray(x, dtype=np.float32)
    w_qkv = np.asarray(w_qkv, dtype=np.float32)

    if _CACHED_NC is None:
        _CACHED_NC = build_kernel()
    in_maps = make_in_maps(x, w_qkv)
    res = run_bass_kernel_spmd(_CACHED_NC, in_maps, core_ids=list(range(NCORES)))
    out2 = assemble_output(res.results, x)
    return (x, out2)

